# revision 1
# baseline (speedup 1.0000x reference)
"""Trainium2 Bass kernel for nn_InvestigationBlock (dense transformer block).

Block: LN1 -> qkv -> polynomial-softmax attention -> proj -> +residual
       -> LN2 -> fc1 -> PolyGELU -> fc2 -> +residual

Sharding (8 cores, no collectives): core c handles batch b=c//2 and
query-token half s=c%2 (1024 of 2048 tokens). Each core computes k/v for
the full 2048 tokens of its batch element (2x redundancy on the k/v part
of qkv), everything else is computed only for its 1024 query rows. The
final output rows are exact and disjoint across cores; the host just
concatenates.

Layout strategy on-chip:
 - LayerNorms computed token-major ([128 tok, 768]) where mean/rstd are
   per-partition scalars (cheap tensor_scalar apply), output cast to bf16
   and moved to feature-major ([768, N]) via DMA transpose (bf16 XBAR).
 - All GEMMs consume feature-major bf16 activations: out^T = W.T @ actT
   with W (stored [in,out]) as the stationary operand.
 - Attention per head: scores S^T[ktok, q] = k^T.T @ q^T (K=64),
   poly+clamp fused as ACT Square (scale/bias folded) + DVE 2-op
   tensor_scalar (add const, max eps). A@V uses V with an appended
   ones-column so the row-sum r rides along as PSUM row 64; normalize via
   reciprocal + gpsimd partition-broadcast.
 - Residual stream stays fp32 token-major; branch outputs are transposed
   back with PE-transpose (fp32) and fused-added during PSUM evacuation.
 - LN gamma/beta folded into the following matmul's weights/bias on the
   host; per-feature biases folded into ACT evacuation bias vectors.
"""

import os
import sys

for _p in ("/opt/trn_rl_repo", os.path.expanduser("~/.axon_site/_ro/trn_rl_repo")):
    if os.path.isdir(_p) and _p not in sys.path:
        sys.path.insert(0, _p)

import math
from contextlib import ExitStack

import ml_dtypes
import numpy as np

import concourse.bass as bass
import concourse.mybir as mybir
import concourse.tile as tile
from concourse import bacc
from concourse.bass_utils import run_bass_kernel_spmd
from concourse.masks import make_identity

F32 = mybir.dt.float32
BF16 = mybir.dt.bfloat16

DIM = 768
HEADS = 12
HD = 64
HIDDEN = 4 * DIM
NTOK = 2048
NQ = 1024
NB = 4
SCALE = HD ** -0.5
LN_EPS = 1e-5
P = 128

KC = DIM // P          # 6 contraction chunks for DIM
TC_KV = NTOK // P      # 16 token tiles (kv)
TC_Q = NQ // P         # 8 token tiles (q)
QCH = NQ // 512        # 2 query chunks of 512
MC_H = HIDDEN // P     # 24 feature chunks of hidden


def _f(x):
    return float(np.asarray(x))


class Cfg:
    """Host-folded constants baked into the program."""

    def __init__(self, inputs):
        a, b, c = _f(inputs["attn_a"]), _f(inputs["attn_b"]), _f(inputs["attn_c"])
        ga, gb, gc = _f(inputs["gelu_a"]), _f(inputs["gelu_b"]), _f(inputs["gelu_c"])
        assert a > 0 and ga > 0
        # a*(Sx)^2 + b*(Sx) + c = (sa*S*x + b/(2sa))^2 + (c - b^2/(4a))
        sa = math.sqrt(a)
        self.attn_scale = sa * SCALE
        self.attn_bias = b / (2 * sa)
        self.attn_d = c - b * b / (4 * a)
        sg = math.sqrt(ga)
        self.gelu_scale = sg
        self.gelu_bias0 = gb / (2 * sg)  # bias before adding fc1 bias contribution
        self.gelu_d = gc - gb * gb / (4 * ga)


def build_nc(cfg, qkv_b_eff, proj_b, fc2_b, v_bias_nonzero, qk_bias_nonzero,
             pb_nonzero, f2b_nonzero):
    nc = bacc.Bacc(None, target_bir_lowering=False)

    x_kv = nc.dram_tensor("x_kv", [NTOK, DIM], F32, kind="ExternalInput").ap()
    x_q = nc.dram_tensor("x_q", [NQ, DIM], F32, kind="ExternalInput").ap()
    w_qkv = nc.dram_tensor("w_qkv", [DIM, 3 * DIM], BF16, kind="ExternalInput").ap()
    w_proj = nc.dram_tensor("w_proj", [DIM, DIM], BF16, kind="ExternalInput").ap()
    w_fc1 = nc.dram_tensor("w_fc1", [DIM, HIDDEN], BF16, kind="ExternalInput").ap()
    w_fc2 = nc.dram_tensor("w_fc2", [HIDDEN, DIM], BF16, kind="ExternalInput").ap()
    # per-out-feature bias vectors (fp32), stored as [chunks, 128]
    b_qk = nc.dram_tensor("b_qk", [2 * KC, P], F32, kind="ExternalInput").ap()
    b_v = nc.dram_tensor("b_v", [DIM], F32, kind="ExternalInput").ap()
    b_proj = nc.dram_tensor("b_proj", [KC, P], F32, kind="ExternalInput").ap()
    b_fc2 = nc.dram_tensor("b_fc2", [KC, P], F32, kind="ExternalInput").ap()
    b_gelu = nc.dram_tensor("b_gelu", [MC_H, P], F32, kind="ExternalInput").ap()
    y = nc.dram_tensor("y", [NQ, DIM], F32, kind="ExternalOutput").ap()

    with tile.TileContext(nc) as tc, ExitStack() as ctx:
        singles = ctx.enter_context(tc.tile_pool(name="singles", bufs=1))

        ident = singles.tile([P, P], F32)
        make_identity(nc, ident)

        eps_sb = singles.tile([P, 1], F32)
        nc.vector.memset(eps_sb, LN_EPS)
        ab_sb = singles.tile([P, 1], F32)
        nc.vector.memset(ab_sb, cfg.attn_bias)

        b_qk_sb = singles.tile([P, 2 * KC], F32)
        nc.sync.dma_start(b_qk_sb, b_qk.rearrange("c p -> p c"))
        b_proj_sb = singles.tile([P, KC], F32)
        nc.sync.dma_start(b_proj_sb, b_proj.rearrange("c p -> p c"))
        b_fc2_sb = singles.tile([P, KC], F32)
        nc.sync.dma_start(b_fc2_sb, b_fc2.rearrange("c p -> p c"))
        b_gelu_sb = singles.tile([P, MC_H], F32)
        nc.sync.dma_start(b_gelu_sb, b_gelu.rearrange("c p -> p c"))
        if v_bias_nonzero:
            bv_row = singles.tile([1, DIM], F32)
            nc.sync.dma_start(bv_row, b_v[None, :])
            bv_b = singles.tile([P, DIM], F32)
            nc.gpsimd.partition_broadcast(bv_b, bv_row)

        # residual stream tiles (fp32 token-major); x2 overwrites xq in place
        xq_tiles = [singles.tile([P, DIM], F32, name=f"xq{t}") for t in range(TC_Q)]
        x2_tiles = xq_tiles

        # pool A2: lives through attention + proj
        ctxA2 = ExitStack()
        poolA2 = ctxA2.enter_context(tc.tile_pool(name="poolA2", bufs=1))
        qT = poolA2.tile([P, KC, NQ], BF16, name="qT")
        kT = poolA2.tile([P, KC, NTOK], BF16, name="kT")
        # v token-major with per-head ones column: [ktok, kt, head, 64+1]
        v_sb = poolA2.tile([P, TC_KV, HEADS, HD + 1], BF16, name="v_sb")
        nc.vector.memset(v_sb[:, :, :, HD:HD + 1], 1.0)
        attnT = poolA2.tile([P, KC, NQ], BF16, name="attnT")
        wproj_sb = poolA2.tile([P, KC, DIM], BF16, name="wproj_sb")
        nc.sync.dma_start(wproj_sb, w_proj.rearrange("(c p) o -> p c o", p=P))

        # pool A1: LN1 + qkv only
        ctxA1 = ExitStack()
        poolA1 = ctxA1.enter_context(tc.tile_pool(name="poolA1", bufs=1))
        wqkv_sb = poolA1.tile([P, KC, 3 * DIM], BF16, name="wqkv_sb")
        nc.sync.dma_start(wqkv_sb, w_qkv.rearrange("(c p) o -> p c o", p=P))
        hkvT = poolA1.tile([P, KC, NTOK], BF16, name="hkvT")
        hqT = poolA1.tile([P, KC, NQ], BF16, name="hqT")

        # ---------------- LN1 + transpose to feature-major ----------------
        def ln_tile(pool, src_tile, out_bf):
            """token-major LN: out_bf = (x - mean(x)) * rsqrt(var(x)+eps)."""
            stats = pool.tile([P, 3, 6], F32, tag="stats", name="stats")
            for sg in range(3):
                nc.vector.bn_stats(stats[:, sg], src_tile[:, sg * 256:(sg + 1) * 256])
            mv = pool.tile([P, 2], F32, tag="mv", name="mv")
            nc.vector.bn_aggr(mv, stats)
            rstd = pool.tile([P, 1], F32, tag="rstd", name="rstd")
            nc.scalar.activation(rstd, mv[:, 1:2],
                                 mybir.ActivationFunctionType.Sqrt, bias=eps_sb)
            nc.vector.reciprocal(rstd, rstd)
            nc.vector.tensor_scalar(out_bf, src_tile, mv[:, 0:1], rstd,
                                    mybir.AluOpType.subtract, mybir.AluOpType.mult)

        with tc.tile_pool(name="ln", bufs=3) as ln_pool:
            for t in range(TC_KV):
                xt = ln_pool.tile([P, DIM], F32, tag="xt", name="xt")
                nc.sync.dma_start(xt, x_kv[t * P:(t + 1) * P, :])
                ht = ln_pool.tile([P, DIM], BF16, tag="ht", name="ht")
                ln_tile(ln_pool, xt, ht)
                for fc in range(KC):
                    nc.sync.dma_start_transpose(
                        hkvT[:, fc, t * P:(t + 1) * P], ht[:, fc * P:(fc + 1) * P])
            for t in range(TC_Q):
                nc.sync.dma_start(xq_tiles[t], x_q[t * P:(t + 1) * P, :])
                ht = ln_pool.tile([P, DIM], BF16, tag="ht", name="ht")
                ln_tile(ln_pool, xq_tiles[t], ht)
                for fc in range(KC):
                    nc.sync.dma_start_transpose(
                        hqT[:, fc, t * P:(t + 1) * P], ht[:, fc * P:(fc + 1) * P])

        # ---------------- qkv ----------------
        def evac(dst, src, bias_ap):
            if bias_ap is None:
                nc.scalar.activation(dst, src, mybir.ActivationFunctionType.Copy)
            else:
                nc.scalar.activation(dst, src,
                                     mybir.ActivationFunctionType.Identity,
                                     bias=bias_ap)

        with tc.tile_pool(name="qkv_ps", bufs=3, space="PSUM") as qkv_ps:
            # q^T and k^T (feature-major)
            for dst, rhs, ncols, off in ((qT, hqT, QCH, 0), (kT, hkvT, NTOK // 512, DIM)):
                for mc in range(KC):
                    for qc in range(ncols):
                        pt = qkv_ps.tile([P, 512], F32, tag="mm", name="mm")
                        for kc in range(KC):
                            nc.tensor.matmul(
                                pt,
                                wqkv_sb[:, kc, off + mc * P:off + (mc + 1) * P],
                                rhs[:, kc, qc * 512:(qc + 1) * 512],
                                start=(kc == 0), stop=(kc == KC - 1))
                        bias_ap = None
                        if qk_bias_nonzero:
                            i = (off // DIM) * KC + mc
                            bias_ap = b_qk_sb[:, i:i + 1]
                        evac(dst[:, mc, qc * 512:(qc + 1) * 512], pt, bias_ap)
            # v (token-major, interleaved per-head with ones col)
            for t in range(TC_KV):
                for half in range(2):  # heads 0..7 then 8..11 (512 + 256 cols)
                    ncol = 512 if half == 0 else 256
                    nh = ncol // HD
                    pt = qkv_ps.tile([P, 512], F32, tag="mm", name="pt")[:, :ncol]
                    for kc in range(KC):
                        nc.tensor.matmul(
                            pt,
                            hkvT[:, kc, t * P:(t + 1) * P],
                            wqkv_sb[:, kc, 2 * DIM + half * 512:
                                    2 * DIM + half * 512 + ncol],
                            start=(kc == 0), stop=(kc == KC - 1))
                    h0 = half * 8
                    dst = v_sb[:, t, h0:h0 + nh, 0:HD]
                    src = pt.rearrange("p (h d) -> p h d", d=HD)
                    if v_bias_nonzero:
                        nc.vector.tensor_tensor(
                            dst, src,
                            bv_b[:, half * 512:half * 512 + ncol]
                            .rearrange("p (h d) -> p h d", d=HD),
                            mybir.AluOpType.add)
                    else:
                        nc.scalar.activation(dst, src,
                                             mybir.ActivationFunctionType.Copy)

        ctxA1.close()

        # ---------------- attention ----------------
        with tc.tile_pool(name="at", bufs=3) as at_pool, \
             tc.tile_pool(name="sc_ps", bufs=3, space="PSUM") as sc_ps, \
             tc.tile_pool(name="av_ps", bufs=2, space="PSUM") as av_ps:
            for h in range(HEADS):
                base = (h % 2) * HD
                g = h // 2
                for qc in range(QCH):
                    av = av_ps.tile([HD + 1, 512], F32, tag="av", name="av")
                    for kt in range(TC_KV):
                        st = sc_ps.tile([P, 512], F32, tag="sc", name="sc")
                        nc.tensor.matmul(
                            st,
                            kT[base:base + HD, g, kt * P:(kt + 1) * P],
                            qT[base:base + HD, g, qc * 512:(qc + 1) * 512],
                            start=True, stop=True)
                        at = at_pool.tile([P, 512], BF16, tag="a", name="a")
                        nc.scalar.activation(at, st,
                                             mybir.ActivationFunctionType.Square,
                                             bias=ab_sb,
                                             scale=cfg.attn_scale)
                        nc.vector.tensor_scalar(at, at, cfg.attn_d, 1e-6,
                                                mybir.AluOpType.add,
                                                mybir.AluOpType.max)
                        nc.tensor.matmul(av, v_sb[:, kt, h, :], at,
                                         start=(kt == 0), stop=(kt == TC_KV - 1))
                    # normalize: attn^T[d, q] = av[d, q] / (av[64, q] + 1e-8)
                    rr = at_pool.tile([1, 512], F32, tag="rr", name="rr")
                    nc.scalar.activation(rr, av[HD:HD + 1, :],
                                         mybir.ActivationFunctionType.Copy,
                                         bias=1e-8)
                    nc.vector.reciprocal(rr, rr)
                    rb = at_pool.tile([HD, 512], F32, tag="rb", name="rb")
                    nc.gpsimd.partition_broadcast(rb, rr)
                    nc.vector.tensor_tensor(
                        attnT[base:base + HD, g, qc * 512:(qc + 1) * 512],
                        av[0:HD, :], rb, mybir.AluOpType.mult)

        # ---------------- proj + residual -> x2 ----------------
        with tc.tile_pool(name="pj", bufs=2) as pj_pool, \
             tc.tile_pool(name="pj_ps", bufs=3, space="PSUM") as pj_ps:
            projT = pj_pool.tile([P, KC, NQ], F32, tag="projT", bufs=1, name="projT")
            for mc in range(KC):
                for qc in range(QCH):
                    pt = pj_ps.tile([P, 512], F32, tag="mm", name="mm")
                    for kc in range(KC):
                        nc.tensor.matmul(
                            pt, wproj_sb[:, kc, mc * P:(mc + 1) * P],
                            attnT[:, kc, qc * 512:(qc + 1) * 512],
                            start=(kc == 0), stop=(kc == KC - 1))
                    evac(projT[:, mc, qc * 512:(qc + 1) * 512], pt,
                         b_proj_sb[:, mc:mc + 1] if pb_nonzero else None)
            for t in range(TC_Q):
                for mc in range(KC):
                    tp = pj_ps.tile([P, P], F32, tag="tr", name="tr")
                    nc.tensor.transpose(tp, projT[:, mc, t * P:(t + 1) * P], ident)
                    nc.vector.scalar_tensor_tensor(
                        x2_tiles[t][:, mc * P:(mc + 1) * P], tp, 1.0,
                        xq_tiles[t][:, mc * P:(mc + 1) * P],
                        mybir.AluOpType.mult, mybir.AluOpType.add)

        ctxA2.close()  # release poolA2

        # ---------------- LN2 -> h2^T ----------------
        poolB = ctx.enter_context(tc.tile_pool(name="poolB", bufs=1))
        h2T = poolB.tile([P, KC, NQ], BF16, name="h2T")
        with tc.tile_pool(name="ln2", bufs=3) as ln2_pool:
            for t in range(TC_Q):
                ht = ln2_pool.tile([P, DIM], BF16, tag="ht", name="ht")
                ln_tile(ln2_pool, x2_tiles[t], ht)
                for fc in range(KC):
                    nc.sync.dma_start_transpose(
                        h2T[:, fc, t * P:(t + 1) * P], ht[:, fc * P:(fc + 1) * P])

        # ---------------- MLP + residual -> y ----------------
        with tc.tile_pool(name="mlp", bufs=2) as mlp_pool, \
             tc.tile_pool(name="mlp_ps", bufs=3, space="PSUM") as mlp_ps:
            wfc1_sb = mlp_pool.tile([P, KC, HIDDEN], BF16, tag="wfc1", bufs=1, name="wfc1")
            nc.sync.dma_start(wfc1_sb, w_fc1.rearrange("(c p) o -> p c o", p=P))
            wfc2_sb = mlp_pool.tile([P, MC_H, DIM], BF16, tag="wfc2", bufs=1, name="wfc2")
            nc.sync.dma_start(wfc2_sb, w_fc2.rearrange("(c p) o -> p c o", p=P))
            for qc in range(QCH):
                gT = mlp_pool.tile([P, MC_H, 512], BF16, tag="gT", bufs=2, name="gT")
                for mc in range(MC_H):
                    pt = mlp_ps.tile([P, 512], F32, tag="mm", name="mm")
                    for kc in range(KC):
                        nc.tensor.matmul(
                            pt, wfc1_sb[:, kc, mc * P:(mc + 1) * P],
                            h2T[:, kc, qc * 512:(qc + 1) * 512],
                            start=(kc == 0), stop=(kc == KC - 1))
                    # PolyGELU: Square(sg*u + bias_vec) + gelu_d
                    nc.scalar.activation(gT[:, mc], pt,
                                         mybir.ActivationFunctionType.Square,
                                         bias=b_gelu_sb[:, mc:mc + 1],
                                         scale=cfg.gelu_scale)
                    nc.vector.tensor_scalar_add(gT[:, mc], gT[:, mc], cfg.gelu_d)
                f2T = mlp_pool.tile([P, KC, 512], F32, tag="f2T", bufs=2, name="f2T")
                for mc in range(KC):
                    pt = mlp_ps.tile([P, 512], F32, tag="mm", name="mm")
                    for kc in range(MC_H):
                        nc.tensor.matmul(
                            pt, wfc2_sb[:, kc, mc * P:(mc + 1) * P],
                            gT[:, kc, :],
                            start=(kc == 0), stop=(kc == MC_H - 1))
                    evac(f2T[:, mc], pt,
                         b_fc2_sb[:, mc:mc + 1] if f2b_nonzero else None)
                for qt in range(4):
                    t = qc * 4 + qt
                    yt = mlp_pool.tile([P, DIM], F32, tag="yt", bufs=2, name="yt")
                    for mc in range(KC):
                        tp = mlp_ps.tile([P, P], F32, tag="tr", name="tr")
                        nc.tensor.transpose(tp, f2T[:, mc, qt * P:(qt + 1) * P],
                                            ident)
                        nc.vector.scalar_tensor_tensor(
                            yt[:, mc * P:(mc + 1) * P], tp, 1.0,
                            x2_tiles[t][:, mc * P:(mc + 1) * P],
                            mybir.AluOpType.mult, mybir.AluOpType.add)
                    nc.sync.dma_start(y[t * P:(t + 1) * P, :], yt)

    nc.compile()
    return nc


_CACHED = {}


def kernel(**inputs) -> np.ndarray:
    ins = {k: np.asarray(v) for k, v in inputs.items()}
    x = ins["x"].astype(np.float32)
    cfg = Cfg(ins)

    ln1_g, ln1_b = ins["ln1_g"].astype(np.float32), ins["ln1_b"].astype(np.float32)
    ln2_g, ln2_b = ins["ln2_g"].astype(np.float32), ins["ln2_b"].astype(np.float32)
    qkv_w = ins["qkv_w"].astype(np.float32)
    fc1_w = ins["fc1_w"].astype(np.float32)

    qkv_w_eff = ln1_g[:, None] * qkv_w
    qkv_b_eff = ins["qkv_b"].astype(np.float32) + ln1_b @ qkv_w
    fc1_w_eff = ln2_g[:, None] * fc1_w
    fc1_b_eff = ins["fc1_b"].astype(np.float32) + ln2_b @ fc1_w

    b_qk = qkv_b_eff[:2 * DIM]
    b_v = qkv_b_eff[2 * DIM:]
    b_proj = ins["proj_b"].astype(np.float32)
    b_fc2 = ins["fc2_b"].astype(np.float32)
    # fc1 bias folded into the gelu ACT bias vector:
    # Square(sg*u + (sg*b + gb/(2sg))) + d
    b_gelu = cfg.gelu_scale * fc1_b_eff + cfg.gelu_bias0

    qk_bias_nonzero = bool(np.any(b_qk != 0.0))
    v_bias_nonzero = bool(np.any(b_v != 0.0))
    pb_nonzero = bool(np.any(b_proj != 0.0))
    f2b_nonzero = bool(np.any(b_fc2 != 0.0))

    key = (qk_bias_nonzero, v_bias_nonzero, pb_nonzero, f2b_nonzero,
           cfg.attn_scale, cfg.attn_bias, cfg.attn_d,
           cfg.gelu_scale, cfg.gelu_d)
    if key not in _CACHED:
        _CACHED[key] = build_nc(cfg, qkv_b_eff, b_proj, b_fc2, v_bias_nonzero,
                                qk_bias_nonzero, pb_nonzero, f2b_nonzero)
    nc = _CACHED[key]

    bf = ml_dtypes.bfloat16
    common = {
        "w_qkv": np.ascontiguousarray(qkv_w_eff.astype(bf)),
        "w_proj": np.ascontiguousarray(ins["proj_w"].astype(np.float32).astype(bf)),
        "w_fc1": np.ascontiguousarray(fc1_w_eff.astype(bf)),
        "w_fc2": np.ascontiguousarray(ins["fc2_w"].astype(np.float32).astype(bf)),
        "b_qk": np.ascontiguousarray(b_qk.reshape(2 * KC, P)),
        "b_v": np.ascontiguousarray(b_v),
        "b_proj": np.ascontiguousarray(b_proj.reshape(KC, P)),
        "b_fc2": np.ascontiguousarray(b_fc2.reshape(KC, P)),
        "b_gelu": np.ascontiguousarray(b_gelu.reshape(MC_H, P)),
    }
    in_maps = []
    for c in range(8):
        b, s = c // 2, c % 2
        m = dict(common)
        m["x_kv"] = np.ascontiguousarray(x[b])
        m["x_q"] = np.ascontiguousarray(x[b, s * NQ:(s + 1) * NQ])
        in_maps.append(m)

    res = run_bass_kernel_spmd(nc, in_maps, core_ids=list(range(8)))

    out = np.empty((NB, NTOK, DIM), dtype=np.float32)
    for c in range(8):
        b, s = c // 2, c % 2
        out[b, s * NQ:(s + 1) * NQ] = res.results[c]["y"]
    return out


if __name__ == "__main__":
    rng = np.random.default_rng(0)
    fake = {
        "x": rng.standard_normal((NB, NTOK, DIM), dtype=np.float32),
    }
    print("use test.py instead")



# revision 11
# speedup vs baseline: 1.7317x; 1.7317x over previous
"""Trainium2 Bass kernel for nn_InvestigationBlock (dense transformer block).

Block: LN1 -> qkv -> polynomial-softmax attention -> proj -> +residual
       -> LN2 -> fc1 -> PolyGELU -> fc2 -> +residual

Sharding (8 cores, no collectives): core c handles batch b=c//2 and
query-token half s=c%2 (1024 of 2048 tokens). Each core computes k/v for
the full 2048 tokens of its batch element, everything else only for its
1024 query rows. Output rows are exact and disjoint across cores.

v1 structure (vs v0 baseline):
 - LN1 fused with qkv per 512-token group; h^T built with one 3D-output
   DMA transpose per token tile; q-half LN reuses hkvT (no separate hqT).
 - Bias vectors passed host-pretransposed [128, C] (contiguous DMA).
 - Weights DMA'd on the scalar-engine HWDGE queue; activations/x on sync.
 - fc1/fc2 weights prefetched during attention (after qkv weights die).
 - Scores: head pairs (2g, 2g+1) issued back-to-back as K=64 row-tiled
   matmuls at partition bases 0/64 -> concurrent execution on the PE.
 - Score poly split between ACT path (Square) and DVE path (affine+mul)
   to balance engine load; clamp fused as 2-op tensor_scalar.
 - Normalize: row-sums ride in v's ones-column; reciprocal batched as
   [8, 256] per head pair; 1/r broadcast to 64 partitions via tiny
   masked matmuls (no gpsimd, no [1,512] reciprocals).
"""

import os
import sys

for _p in ("/opt/trn_rl_repo", os.path.expanduser("~/.axon_site/_ro/trn_rl_repo")):
    if os.path.isdir(_p) and _p not in sys.path:
        sys.path.insert(0, _p)

import math
from contextlib import ExitStack

import ml_dtypes
import numpy as np

import concourse.bass as bass
import concourse.mybir as mybir
import concourse.tile as tile
from concourse import bacc
from concourse.bass_utils import run_bass_kernel_spmd
from concourse.masks import make_identity

F32 = mybir.dt.float32
BF16 = mybir.dt.bfloat16

DIM = 768
HEADS = 12
HD = 64
HIDDEN = 4 * DIM
NTOK = 2048
NQ = 1024
NB = 4
SCALE = HD ** -0.5
LN_EPS = 1e-5
P = 128

KC = DIM // P          # 6 contraction chunks for DIM
TC_KV = NTOK // P      # 16 token tiles (kv)
TC_Q = NQ // P         # 8 token tiles (q)
QCH = NQ // 512        # 2 query chunks of 512
MC_H = HIDDEN // P     # 24 feature chunks of hidden
NG_KV = NTOK // 512    # 4 kv token groups of 512
HP = HEADS // 2        # 6 head pairs

# which (parity, qc) score tiles take the DVE path (rest go ACT Square)
DVE_TILES = ((1, 1),)


def _f(x):
    return float(np.asarray(x))


class Cfg:
    """Host-folded constants baked into the program."""

    def __init__(self, inputs):
        a, b, c = _f(inputs["attn_a"]), _f(inputs["attn_b"]), _f(inputs["attn_c"])
        ga, gb, gc = _f(inputs["gelu_a"]), _f(inputs["gelu_b"]), _f(inputs["gelu_c"])
        assert a > 0 and ga > 0
        # a*(Sx)^2 + b*(Sx) + c = (sa*S*x + b/(2sa))^2 + (c - b^2/(4a))
        sa = math.sqrt(a)
        self.attn_scale = sa * SCALE
        self.attn_bias = b / (2 * sa)
        self.attn_d = c - b * b / (4 * a)
        sg = math.sqrt(ga)
        self.gelu_scale = sg
        self.gelu_bias0 = gb / (2 * sg)
        self.gelu_d = gc - gb * gb / (4 * ga)


def build_nc(cfg, v_bias_nonzero, qk_bias_nonzero, pb_nonzero, f2b_nonzero):
    nc = bacc.Bacc(None, target_bir_lowering=False)

    x_kv = nc.dram_tensor("x_kv", [NTOK, DIM], F32, kind="ExternalInput").ap()
    w_qkv = nc.dram_tensor("w_qkv", [DIM, 3 * DIM], BF16, kind="ExternalInput").ap()
    w_proj = nc.dram_tensor("w_proj", [DIM, DIM], BF16, kind="ExternalInput").ap()
    w_fc1 = nc.dram_tensor("w_fc1", [DIM, HIDDEN], BF16, kind="ExternalInput").ap()
    w_fc2 = nc.dram_tensor("w_fc2", [HIDDEN, DIM], BF16, kind="ExternalInput").ap()
    # per-out-feature bias vectors (fp32), host-pretransposed to [128, C]
    b_qk = nc.dram_tensor("b_qk", [P, 2 * KC], F32, kind="ExternalInput").ap()
    b_v = nc.dram_tensor("b_v", [DIM], F32, kind="ExternalInput").ap()
    b_proj = nc.dram_tensor("b_proj", [P, KC], F32, kind="ExternalInput").ap()
    b_fc2 = nc.dram_tensor("b_fc2", [P, KC], F32, kind="ExternalInput").ap()
    b_gelu = nc.dram_tensor("b_gelu", [P, MC_H], F32, kind="ExternalInput").ap()
    y = nc.dram_tensor("y", [NQ, DIM], F32, kind="ExternalOutput").ap()

    # host reorders x_kv so the q half is always token tiles [0, TC_Q);
    # attention sums over key tokens are permutation-invariant.
    q_t0 = 0

    with tile.TileContext(nc) as tc, ExitStack() as ctx:
        singles = ctx.enter_context(tc.tile_pool(name="singles", bufs=1))

        ident = singles.tile([P, P], F32)
        make_identity(nc, ident)

        eps_sb = singles.tile([P, 1], F32)
        nc.vector.memset(eps_sb, LN_EPS)
        ab_sb = singles.tile([P, 1], F32)
        nc.vector.memset(ab_sb, cfg.attn_bias)
        # mask4[32k, k*64:(k+1)*64] = 1 -> K=128 matmul broadcasts row 32k
        # of the reciprocal staging tile to 64 output partitions
        mask4 = singles.tile([P, 4 * HD], BF16)
        nc.vector.memset(mask4, 0.0)
        for k in range(4):
            nc.vector.memset(mask4[32 * k:32 * k + 1, k * HD:(k + 1) * HD], 1.0)
        rtmp = singles.tile([P, 512], F32)
        nc.vector.memset(rtmp, 1.0)

        b_qk_sb = singles.tile([P, 2 * KC], F32)
        nc.scalar.dma_start(b_qk_sb, b_qk)
        b_proj_sb = singles.tile([P, KC], F32)
        nc.scalar.dma_start(b_proj_sb, b_proj)
        b_fc2_sb = singles.tile([P, KC], F32)
        nc.scalar.dma_start(b_fc2_sb, b_fc2)
        b_gelu_sb = singles.tile([P, MC_H], F32)
        nc.scalar.dma_start(b_gelu_sb, b_gelu)
        if v_bias_nonzero:
            bv_row = singles.tile([1, DIM], F32)
            nc.scalar.dma_start(bv_row, b_v[None, :])
            bv_b = singles.tile([P, DIM], F32)
            nc.gpsimd.partition_broadcast(bv_b, bv_row)

        # residual stream tiles (fp32 token-major), filled during LN1 loads
        xq_tiles = [singles.tile([P, DIM], F32, name=f"xq{t}") for t in range(TC_Q)]
        x2_tiles = xq_tiles

        # fc1 weights: slot reserved up front (outer pool), DMA issued after
        # the qkv weights die so the load overlaps attention
        poolW = ctx.enter_context(tc.tile_pool(name="poolW", bufs=1))
        wfc1_sb = poolW.tile([P, KC, HIDDEN], BF16, name="wfc1")

        # ---------- pool A2: lives through attention + proj ----------
        ctxA2 = ExitStack()
        poolA2 = ctxA2.enter_context(tc.tile_pool(name="poolA2", bufs=1))
        qT = poolA2.tile([P, KC, NQ], BF16, name="qT")
        kT = poolA2.tile([P, KC, NTOK], BF16, name="kT")
        v_sb = poolA2.tile([P, TC_KV, HEADS, HD + 1], BF16, name="v_sb")
        nc.vector.memset(v_sb[:, :, :, HD:HD + 1], 1.0)

        # ---------- pool A1: LN1 + qkv only ----------
        ctxA1 = ExitStack()
        poolA1 = ctxA1.enter_context(tc.tile_pool(name="poolA1", bufs=1))
        wqkv_sb = poolA1.tile([P, KC, 3 * DIM], BF16, name="wqkv_sb")
        nc.scalar.dma_start(wqkv_sb, w_qkv.rearrange("(c p) o -> p c o", p=P))
        hkvT = poolA1.tile([P, KC, NTOK], BF16, name="hkvT")

        def ln_tile(pool, src_tile, out_bf):
            """token-major LN: out_bf = (x - mean(x)) * rsqrt(var(x)+eps)."""
            stats = pool.tile([P, 3, 6], F32, tag="stats", name="stats")
            for sg in range(3):
                nc.vector.bn_stats(stats[:, sg], src_tile[:, sg * 256:(sg + 1) * 256])
            mv = pool.tile([P, 2], F32, tag="mv", name="mv")
            nc.vector.bn_aggr(mv, stats)
            rstd = pool.tile([P, 1], F32, tag="rstd", name="rstd")
            nc.scalar.activation(rstd, mv[:, 1:2],
                                 mybir.ActivationFunctionType.Sqrt, bias=eps_sb)
            nc.vector.reciprocal(rstd, rstd)
            nc.vector.tensor_scalar(out_bf, src_tile, mv[:, 0:1], rstd,
                                    mybir.AluOpType.subtract, mybir.AluOpType.mult)

        def evac(dst, src, bias_ap):
            if bias_ap is None:
                nc.scalar.activation(dst, src, mybir.ActivationFunctionType.Copy)
            else:
                nc.scalar.activation(dst, src,
                                     mybir.ActivationFunctionType.Identity,
                                     bias=bias_ap)

        # ---------- LN1 + qkv, interleaved per 512-token group ----------
        with tc.tile_pool(name="ln", bufs=3) as ln_pool, \
             tc.tile_pool(name="qkv_ps", bufs=3, space="PSUM") as qkv_ps:
            for g in range(NG_KV):
                # LN + transpose the 4 token tiles of this group
                for ti in range(4):
                    t = g * 4 + ti
                    if q_t0 <= t < q_t0 + TC_Q:
                        xt = xq_tiles[t - q_t0]
                    else:
                        xt = ln_pool.tile([P, DIM], F32, tag="xt", name="xt")
                    nc.sync.dma_start(xt, x_kv[t * P:(t + 1) * P, :])
                    ht = ln_pool.tile([P, DIM], BF16, tag="ht", name="ht")
                    ln_tile(ln_pool, xt, ht)
                    nc.sync.dma_start_transpose(
                        hkvT[:, :, t * P:(t + 1) * P], ht)
                gs = slice(g * 512, (g + 1) * 512)
                # k^T for this group's 512 tokens
                for mc in range(KC):
                    pt = qkv_ps.tile([P, 512], F32, tag="mm", name="mm")
                    for kc in range(KC):
                        nc.tensor.matmul(
                            pt,
                            wqkv_sb[:, kc, DIM + mc * P:DIM + (mc + 1) * P],
                            hkvT[:, kc, gs],
                            start=(kc == 0), stop=(kc == KC - 1))
                    bias_ap = b_qk_sb[:, KC + mc:KC + mc + 1] if qk_bias_nonzero else None
                    evac(kT[:, mc, gs], pt, bias_ap)
                # v (token-major, per-head with ones col) for this group
                for ti in range(4):
                    t = g * 4 + ti
                    for half in range(2):
                        ncol = 512 if half == 0 else 256
                        nh = ncol // HD
                        pt = qkv_ps.tile([P, 512], F32, tag="mm", name="pt")[:, :ncol]
                        for kc in range(KC):
                            nc.tensor.matmul(
                                pt,
                                hkvT[:, kc, t * P:(t + 1) * P],
                                wqkv_sb[:, kc, 2 * DIM + half * 512:
                                        2 * DIM + half * 512 + ncol],
                                start=(kc == 0), stop=(kc == KC - 1))
                        h0 = half * 8
                        dst = v_sb[:, t, h0:h0 + nh, 0:HD]
                        src = pt.rearrange("p (h d) -> p h d", d=HD)
                        if v_bias_nonzero:
                            nc.vector.tensor_tensor(
                                dst, src,
                                bv_b[:, half * 512:half * 512 + ncol]
                                .rearrange("p (h d) -> p h d", d=HD),
                                mybir.AluOpType.add)
                        else:
                            nc.scalar.activation(dst, src,
                                                 mybir.ActivationFunctionType.Copy)
                # q^T if this group is in the q half
                if q_t0 * P <= g * 512 < (q_t0 + TC_Q) * P:
                    qs = slice(g * 512 - q_t0 * P, g * 512 - q_t0 * P + 512)
                    for mc in range(KC):
                        pt = qkv_ps.tile([P, 512], F32, tag="mm", name="mm")
                        for kc in range(KC):
                            nc.tensor.matmul(
                                pt,
                                wqkv_sb[:, kc, mc * P:(mc + 1) * P],
                                hkvT[:, kc, gs],
                                start=(kc == 0), stop=(kc == KC - 1))
                        bias_ap = b_qk_sb[:, mc:mc + 1] if qk_bias_nonzero else None
                        evac(qT[:, mc, qs], pt, bias_ap)

        ctxA1.close()  # release hkvT + wqkv
        # prefetch fc1 weights during attention (slot was reserved up front)
        nc.scalar.dma_start(wfc1_sb, w_fc1.rearrange("(c p) o -> p c o", p=P))

        # ---------------- attention + proj ----------------
        ctxAt = ExitStack()
        poolAt = ctxAt.enter_context(tc.tile_pool(name="poolAt", bufs=1))
        attnT = poolAt.tile([P, KC, NQ], BF16, name="attnT")
        wproj_sb = poolAt.tile([P, KC, DIM], BF16, name="wproj_sb")
        nc.scalar.dma_start(wproj_sb, w_proj.rearrange("(c p) o -> p c o", p=P))

        with tc.tile_pool(name="at", bufs=4) as at_pool, \
             tc.tile_pool(name="sc_ps", bufs=3, space="PSUM") as sc_ps, \
             tc.tile_pool(name="av_ps", bufs=5, space="PSUM") as av_ps:
            for g in range(HP):
                # 4 accumulators: (parity, qc)
                av = {}
                for par in range(2):
                    for qc in range(QCH):
                        av[(par, qc)] = av_ps.tile([HD + 1, 512], F32,
                                                   tag="av", name="av")
                for kt in range(TC_KV):
                    for qc in range(QCH):
                        sts = {}
                        for par in range(2):
                            st = sc_ps.tile([P, 512], F32, tag="st", name="st")
                            base = par * HD
                            nc.tensor.matmul(
                                st,
                                kT[base:base + HD, g, kt * P:(kt + 1) * P],
                                qT[base:base + HD, g, qc * 512:(qc + 1) * 512],
                                start=True, stop=True)
                            sts[par] = st
                        ats = {}
                        for par in range(2):
                            at = at_pool.tile([P, 512], BF16, tag="a", name="a")
                            if (par, qc) in DVE_TILES:
                                u = at_pool.tile([P, 512], BF16, tag="u", name="u")
                                nc.vector.tensor_scalar(
                                    u, sts[par], cfg.attn_scale, cfg.attn_bias,
                                    mybir.AluOpType.mult, mybir.AluOpType.add)
                                nc.vector.tensor_tensor(at, u, u,
                                                        mybir.AluOpType.mult)
                            else:
                                nc.scalar.activation(
                                    at, sts[par],
                                    mybir.ActivationFunctionType.Square,
                                    bias=ab_sb, scale=cfg.attn_scale)
                            nc.vector.tensor_scalar(at, at, cfg.attn_d, 1e-6,
                                                    mybir.AluOpType.add,
                                                    mybir.AluOpType.max)
                            ats[par] = at
                        for par in range(2):
                            h = 2 * g + par
                            nc.tensor.matmul(av[(par, qc)],
                                             v_sb[:, kt, h, :], ats[par],
                                             start=(kt == 0),
                                             stop=(kt == TC_KV - 1))
                # normalize: attnT[d, q] = av[d, q] * (1 / av[64, q])
                for par in range(2):
                    for qc in range(QCH):
                        row = 32 * (2 * par + qc)
                        nc.scalar.activation(
                            rtmp[row:row + 1, :],
                            av[(par, qc)][HD:HD + 1, :],
                            mybir.ActivationFunctionType.Copy)
                rinv = at_pool.tile([P, 512], BF16, tag="ri", name="ri")
                with nc.allow_low_precision(reason="1/r for attention row "
                                            "normalize; 0.4% on a small branch"):
                    nc.vector.reciprocal(rinv, rtmp)
                for par in range(2):
                    base = par * HD
                    for qc in range(QCH):
                        idx = 2 * par + qc
                        rb = av_ps.tile([HD, 512], F32, tag="av", name="rb")
                        nc.tensor.matmul(
                            rb, mask4[:, idx * HD:(idx + 1) * HD], rinv,
                            start=True, stop=True)
                        rbs = at_pool.tile([HD, 512], F32, tag="rb", name="rbs")
                        nc.scalar.activation(rbs, rb,
                                             mybir.ActivationFunctionType.Copy)
                        nc.vector.tensor_tensor(
                            attnT[base:base + HD, g, qc * 512:(qc + 1) * 512],
                            av[(par, qc)][0:HD, :], rbs, mybir.AluOpType.mult)

        # ---------------- proj + residual -> x2 ----------------
        with tc.tile_pool(name="pj", bufs=2) as pj_pool, \
             tc.tile_pool(name="pj_ps", bufs=3, space="PSUM") as pj_ps:
            projT = pj_pool.tile([P, KC, NQ], F32, tag="projT", bufs=1, name="projT")
            for mc in range(KC):
                for qc in range(QCH):
                    pt = pj_ps.tile([P, 512], F32, tag="mm", name="mm")
                    for kc in range(KC):
                        nc.tensor.matmul(
                            pt, wproj_sb[:, kc, mc * P:(mc + 1) * P],
                            attnT[:, kc, qc * 512:(qc + 1) * 512],
                            start=(kc == 0), stop=(kc == KC - 1))
                    evac(projT[:, mc, qc * 512:(qc + 1) * 512], pt,
                         b_proj_sb[:, mc:mc + 1] if pb_nonzero else None)
            for t in range(TC_Q):
                for mc in range(KC):
                    tp = pj_ps.tile([P, P], F32, tag="tr", name="tr")
                    nc.tensor.transpose(tp, projT[:, mc, t * P:(t + 1) * P], ident)
                    nc.vector.scalar_tensor_tensor(
                        x2_tiles[t][:, mc * P:(mc + 1) * P], tp, 1.0,
                        xq_tiles[t][:, mc * P:(mc + 1) * P],
                        mybir.AluOpType.mult, mybir.AluOpType.add)

        ctxAt.close()  # release attnT/wproj/projT
        ctxA2.close()  # release qT/kT/v_sb

        # ---------------- LN2 -> h2^T (+ fc2 weight prefetch) ----------------
        poolB = ctx.enter_context(tc.tile_pool(name="poolB", bufs=1))
        h2T = poolB.tile([P, KC, NQ], BF16, name="h2T")
        wfc2_sb = poolB.tile([P, MC_H, DIM], BF16, name="wfc2")
        nc.scalar.dma_start(wfc2_sb, w_fc2.rearrange("(c p) o -> p c o", p=P))
        with tc.tile_pool(name="ln2", bufs=3) as ln2_pool:
            for t in range(TC_Q):
                ht = ln2_pool.tile([P, DIM], BF16, tag="ht", name="ht")
                ln_tile(ln2_pool, x2_tiles[t], ht)
                nc.sync.dma_start_transpose(h2T[:, :, t * P:(t + 1) * P], ht)

        # ---------------- MLP + residual -> y ----------------
        with tc.tile_pool(name="mlp", bufs=2) as mlp_pool, \
             tc.tile_pool(name="mlp_ps", bufs=3, space="PSUM") as mlp_ps:
            for qc in range(QCH):
                gT = mlp_pool.tile([P, MC_H, 512], BF16, tag="gT", bufs=2, name="gT")
                for mc in range(MC_H):
                    pt = mlp_ps.tile([P, 512], F32, tag="mm", name="mm")
                    for kc in range(KC):
                        nc.tensor.matmul(
                            pt, wfc1_sb[:, kc, mc * P:(mc + 1) * P],
                            h2T[:, kc, qc * 512:(qc + 1) * 512],
                            start=(kc == 0), stop=(kc == KC - 1))
                    # PolyGELU: Square(sg*u + bias_vec) + gelu_d
                    nc.scalar.activation(gT[:, mc], pt,
                                         mybir.ActivationFunctionType.Square,
                                         bias=b_gelu_sb[:, mc:mc + 1],
                                         scale=cfg.gelu_scale)
                    nc.vector.tensor_scalar_add(gT[:, mc], gT[:, mc], cfg.gelu_d)
                f2T = mlp_pool.tile([P, KC, 512], F32, tag="f2T", bufs=2, name="f2T")
                for mc in range(KC):
                    pt = mlp_ps.tile([P, 512], F32, tag="mm", name="mm")
                    for kc in range(MC_H):
                        nc.tensor.matmul(
                            pt, wfc2_sb[:, kc, mc * P:(mc + 1) * P],
                            gT[:, kc, :],
                            start=(kc == 0), stop=(kc == MC_H - 1))
                    evac(f2T[:, mc], pt,
                         b_fc2_sb[:, mc:mc + 1] if f2b_nonzero else None)
                for qt in range(4):
                    t = qc * 4 + qt
                    yt = mlp_pool.tile([P, DIM], F32, tag="yt", bufs=2, name="yt")
                    for mc in range(KC):
                        tp = mlp_ps.tile([P, P], F32, tag="tr", name="tr")
                        nc.tensor.transpose(tp, f2T[:, mc, qt * P:(qt + 1) * P],
                                            ident)
                        nc.vector.scalar_tensor_tensor(
                            yt[:, mc * P:(mc + 1) * P], tp, 1.0,
                            x2_tiles[t][:, mc * P:(mc + 1) * P],
                            mybir.AluOpType.mult, mybir.AluOpType.add)
                    nc.sync.dma_start(y[t * P:(t + 1) * P, :], yt)

    nc.compile()
    return nc


_CACHED = {}


def build_common_and_cfg(ins):
    cfg = Cfg(ins)
    ln1_g, ln1_b = ins["ln1_g"].astype(np.float32), ins["ln1_b"].astype(np.float32)
    ln2_g, ln2_b = ins["ln2_g"].astype(np.float32), ins["ln2_b"].astype(np.float32)
    qkv_w = ins["qkv_w"].astype(np.float32)
    fc1_w = ins["fc1_w"].astype(np.float32)

    qkv_w_eff = ln1_g[:, None] * qkv_w
    qkv_b_eff = ins["qkv_b"].astype(np.float32) + ln1_b @ qkv_w
    fc1_w_eff = ln2_g[:, None] * fc1_w
    fc1_b_eff = ins["fc1_b"].astype(np.float32) + ln2_b @ fc1_w

    b_qk = qkv_b_eff[:2 * DIM]
    b_v = qkv_b_eff[2 * DIM:]
    b_proj = ins["proj_b"].astype(np.float32)
    b_fc2 = ins["fc2_b"].astype(np.float32)
    b_gelu = cfg.gelu_scale * fc1_b_eff + cfg.gelu_bias0

    bf = ml_dtypes.bfloat16
    common = {
        "w_qkv": np.ascontiguousarray(qkv_w_eff.astype(bf)),
        "w_proj": np.ascontiguousarray(ins["proj_w"].astype(np.float32).astype(bf)),
        "w_fc1": np.ascontiguousarray(fc1_w_eff.astype(bf)),
        "w_fc2": np.ascontiguousarray(ins["fc2_w"].astype(np.float32).astype(bf)),
        "b_qk": np.ascontiguousarray(b_qk.reshape(2 * KC, P).T),
        "b_v": np.ascontiguousarray(b_v),
        "b_proj": np.ascontiguousarray(b_proj.reshape(KC, P).T),
        "b_fc2": np.ascontiguousarray(b_fc2.reshape(KC, P).T),
        "b_gelu": np.ascontiguousarray(b_gelu.reshape(MC_H, P).T),
    }
    flags = (bool(np.any(b_qk != 0.0)), bool(np.any(b_v != 0.0)),
             bool(np.any(b_proj != 0.0)), bool(np.any(b_fc2 != 0.0)))
    return cfg, common, flags


def build_in_maps(ins):
    cfg, common, flags = build_common_and_cfg(ins)
    x = ins["x"].astype(np.float32)
    in_maps = []
    for c in range(8):
        b, s = c // 2, c % 2
        m = dict(common)
        # q half first, other half after (kv order is irrelevant to attention)
        m["x_kv"] = np.ascontiguousarray(
            np.concatenate([x[b, s * NQ:(s + 1) * NQ],
                            x[b, (1 - s) * NQ:(2 - s) * NQ]]))
        in_maps.append(m)
    return cfg, flags, in_maps


def kernel(**inputs) -> np.ndarray:
    ins = {k: np.asarray(v) for k, v in inputs.items()}
    cfg, flags, in_maps = build_in_maps(ins)
    qk_bias_nonzero, v_bias_nonzero, pb_nonzero, f2b_nonzero = flags

    key = (*flags, cfg.attn_scale, cfg.attn_bias, cfg.attn_d,
           cfg.gelu_scale, cfg.gelu_d)
    if key not in _CACHED:
        _CACHED[key] = build_nc(cfg, v_bias_nonzero, qk_bias_nonzero,
                                pb_nonzero, f2b_nonzero)
    nc = _CACHED[key]

    res = run_bass_kernel_spmd(nc, in_maps, core_ids=list(range(8)))

    out = np.empty((NB, NTOK, DIM), dtype=np.float32)
    for c in range(8):
        b, s = c // 2, c % 2
        out[b, s * NQ:(s + 1) * NQ] = res.results[c]["y"]
    return out


if __name__ == "__main__":
    print("use test.py instead")
